# revision 1
# baseline (speedup 1.0000x reference)
"""Trainium2 Bass kernel for nn_Attention_10771777978404 (sparse_attention).

Head-parallel (tensor parallel) sharding over 8 NeuronCores:
  - each core owns NH/8 = 2 heads: computes its q/k/v projections (columns of
    wq/wk/wv), RoPE, causal attention with the low-rank sigmoid gate, and the
    per-head attention outputs (transposed, [d, tok]).
  - the rank-32 adapter (gate) weights are replicated; each core computes the
    full [S, S]-gate implicitly, tile by tile, fused into the attention loop.
  - per-head outputs are AllGathered (bf16) across cores; each core then
    computes a 256-column slice of the final `out @ wo.T` (row-sharded wo) and
    the host concatenates the 8 output slices.

Everything on-device is bf16 with fp32 PSUM accumulation.

self-contained: hardcodes the problem shapes; only needs `concourse` (on
PYTHONPATH in this container) + jax axon devices.
"""

import math
from dataclasses import dataclass

import numpy as np
import ml_dtypes

import concourse.bass as bass
import concourse.tile as tile
from concourse import bacc
from concourse import mybir
from concourse import bass_utils
from concourse.tile_rust import add_dep_helper

BF16 = mybir.dt.bfloat16
F32 = mybir.dt.float32
AF = mybir.ActivationFunctionType


@dataclass(frozen=True)
class Cfg:
    B: int = 2
    S: int = 2048
    DIM: int = 2048
    NH: int = 16
    HD: int = 128
    RANK: int = 32
    NCORES: int = 8
    QT: int = 512   # query block (free dim of score tiles)
    KT: int = 128   # key block (partition dim of score tiles)

    @property
    def HLOC(self):
        return self.NH // self.NCORES

    @property
    def DH(self):
        return self.HLOC * self.HD  # per-core head-dim span

    @property
    def KTILES(self):
        return self.DIM // 128  # contraction tiles for projections

    @property
    def QTN(self):
        return self.S // self.QT

    @property
    def DIAG(self):
        return self.QT // self.KT  # k-tiles per diagonal band


FULL = Cfg()


def build_nc(cfg: Cfg = FULL, *, use_gate=True, use_rs=True, use_bcast=True,
             use_deps=True, use_recip=True, use_rope=True, use_mask=True):
    c = cfg
    assert c.HD == 128 and c.KT == 128
    nc = bacc.Bacc("TRN2", target_bir_lowering=False, debug=False,
                   num_devices=c.NCORES)

    # ---- kernel I/O ----
    xT = nc.dram_tensor("xT", [c.B, c.DIM, c.S], BF16, kind="ExternalInput")
    wqT = nc.dram_tensor("wqT", [c.DIM, c.DH], BF16, kind="ExternalInput")
    wkT = nc.dram_tensor("wkT", [c.DIM, c.DH], BF16, kind="ExternalInput")
    wvT = nc.dram_tensor("wvT", [c.DIM, c.DH], BF16, kind="ExternalInput")
    # woc^T[d_local, j]: this core's head-rows of wo^T (= wo column slice), so
    # the core emits a full-size PARTIAL of the output projection from its own
    # heads; the host sums the partials across cores (no device collective).
    wocT = nc.dram_tensor("wocT", [c.DH, c.DIM], BF16, kind="ExternalInput")
    waT = nc.dram_tensor("waT", [c.DIM, 2 * c.RANK], BF16, kind="ExternalInput")
    c2d = nc.dram_tensor("c2d", [c.HD, c.S], BF16, kind="ExternalInput")
    s2d = nc.dram_tensor("s2d", [c.HD, c.S], BF16, kind="ExternalInput")
    pswapd = nc.dram_tensor("pswapd", [c.HD, c.HD], BF16, kind="ExternalInput")
    maskdd = nc.dram_tensor("maskdd", [c.DIAG, c.KT, c.QT], BF16, kind="ExternalInput")

    # partial output projection, transposed: pout[j, b*S + t]
    pout = nc.dram_tensor("pout", [c.DIM, c.B * c.S], F32, kind="ExternalOutput")

    # scratch for broadcasting 1/rowsum across partitions (DRAM round-trip)
    rrd = nc.dram_tensor("rrd", [c.B * c.S // c.QT * c.HLOC, c.QT], F32)
    # gate tiles sigmoid(A')[k, q] staged via DRAM so the scalar engine never
    # alternates between the Sigmoid and Exp function tables (1.3us reload)
    TBLK = c.DIAG * c.QTN * (c.QTN + 1) // 2
    gdram = nc.dram_tensor("gdram", [c.B, TBLK, c.KT, c.QT], BF16)

    isqrt = 1.0 / math.sqrt(c.HD)
    NQC = c.DH // 128          # per-core q/k head chunks (= HLOC)

    from contextlib import ExitStack
    with ExitStack() as _ctx:
        tc = _ctx.enter_context(tile.TileContext(nc))
        cst = _ctx.enter_context(tc.tile_pool(name="const", bufs=1))
        xtp = _ctx.enter_context(tc.tile_pool(name="xt", bufs=1))
        qkp = _ctx.enter_context(tc.tile_pool(name="qk", bufs=1))
        vp = _ctx.enter_context(tc.tile_pool(name="vp", bufs=1))
        adp = _ctx.enter_context(tc.tile_pool(name="ap", bufs=1))
        rtp = _ctx.enter_context(tc.tile_pool(name="rope_t", bufs=1))
        gio = _ctx.enter_context(tc.tile_pool(name="gio", bufs=8))
        pge = _ctx.enter_context(tc.tile_pool(name="pge", bufs=6))
        nrm = _ctx.enter_context(tc.tile_pool(name="norm", bufs=1))
        wop = _ctx.enter_context(tc.tile_pool(name="wo_out", bufs=3))
        pp = _ctx.enter_context(tc.tile_pool(name="pp", bufs=2, space="PSUM"))
        psp = _ctx.enter_context(tc.tile_pool(name="ps", bufs=2, space="PSUM"))
        pgp = pp  # gate psum shares the projection/wo psum pool (bank budget)
        pop = _ctx.enter_context(tc.tile_pool(name="po", bufs=2, space="PSUM"))
        prsp = _ctx.enter_context(tc.tile_pool(name="prs", bufs=2, space="PSUM"))
        if True:
            # ---- constants / weights ----
            wq_sb = cst.tile([128, c.KTILES, c.DH], BF16, name="wq_sb")
            wk_sb = cst.tile([128, c.KTILES, c.DH], BF16, name="wk_sb")
            wv_sb = cst.tile([128, c.KTILES, c.DH], BF16, name="wv_sb")
            woc_sb = cst.tile([128, NQC, c.DIM], BF16, name="woc_sb")
            wa_sb = cst.tile([128, c.KTILES, 2 * c.RANK], BF16, name="wa_sb")
            c2_sb = cst.tile([128, c.S], BF16, name="c2_sb")
            s2_sb = cst.tile([128, c.S], BF16, name="s2_sb")
            psw_sb = cst.tile([128, 128], BF16, name="psw_sb")
            mask_sb = cst.tile([128, c.DIAG, c.QT], BF16, name="mask_sb")
            ones_sb = cst.tile([128, 1], BF16, name="ones_sb")

            for w_sb, w_d in ((wq_sb, wqT), (wk_sb, wkT), (wv_sb, wvT)):
                wr = w_d.ap().rearrange("(t p) m -> p t m", p=128)
                for half in range(2):
                    h0 = half * (c.KTILES // 2)
                    nc.sync.dma_start(out=w_sb[:, h0:h0 + c.KTILES // 2, :],
                                      in_=wr[:, h0:h0 + c.KTILES // 2, :])
            wcr = wocT.ap().rearrange("(h p) j -> p h j", p=128)
            for h in range(NQC):
                nc.sync.dma_start(out=woc_sb[:, h, :], in_=wcr[:, h, :])
            nc.sync.dma_start(out=wa_sb, in_=waT.ap().rearrange("(t p) m -> p t m", p=128))
            nc.sync.dma_start(out=c2_sb, in_=c2d.ap())
            nc.sync.dma_start(out=s2_sb, in_=s2d.ap())
            nc.sync.dma_start(out=psw_sb, in_=pswapd.ap())
            nc.sync.dma_start(out=mask_sb, in_=maskdd.ap().rearrange("j p q -> p j q"))
            nc.vector.memset(ones_sb, 1.0)

            last_exp_inst = None
            for b in range(c.B):
                # ---- load x^T for this batch ----
                xt_sb = xtp.tile([128, c.KTILES, c.S], BF16, name="xt_sb", tag="xt")
                xr = xT.ap()[b].rearrange("(t p) n -> p t n", p=128)
                for kt in range(c.KTILES):
                    nc.sync.dma_start(out=xt_sb[:, kt, :], in_=xr[:, kt, :])

                # ---- projections ----
                # adapters first: the gate-phase sigmoids only need aq/ak, so
                # ACT gets work early while the PE grinds through q/k/v
                aq_sb = adp.tile([32, c.S], BF16, name="aq_sb", tag="aq")
                ak_sb = adp.tile([32, c.S], BF16, name="ak_sb", tag="ak")
                for dst, col0 in ((aq_sb, 0), (ak_sb, c.RANK)):
                    for qt in range(c.QTN):
                        psum = pp.tile([c.RANK, c.QT], F32, name="psum_a", tag="pp")
                        for kt in range(c.KTILES):
                            nc.tensor.matmul(
                                psum[:, :],
                                wa_sb[:, kt, col0:col0 + c.RANK],
                                xt_sb[:, kt, qt * c.QT:(qt + 1) * c.QT],
                                start=(kt == 0), stop=(kt == c.KTILES - 1))
                        nc.vector.tensor_copy(dst[:, qt * c.QT:(qt + 1) * c.QT],
                                              psum[:, :])

                # gate tiles: one Sigmoid run per batch on ACT, staged through
                # DRAM (Exp and Sigmoid live in different ACT tables; each
                # switch costs a ~1.3us reload, so sigmoids and exps are kept
                # in separate runs via explicit deps). Emitted right after the
                # adapter projections so ACT has work during q/k/v.
                last_sig_inst = None
                if use_gate:
                    for qt in range(c.QTN):
                        qsl = slice(qt * c.QT, (qt + 1) * c.QT)
                        for kt in range(c.DIAG * (qt + 1)):
                            ksl = slice(kt * c.KT, (kt + 1) * c.KT)
                            off = (qt * (qt + 1) // 2) * c.DIAG + kt
                            pga = pgp.tile([128, c.QT], F32, name="pga", tag="pp")
                            nc.tensor.matmul(pga[:, :], ak_sb[:, ksl], aq_sb[:, qsl],
                                             start=True, stop=True)
                            gout = gio.tile([128, c.QT], BF16, name="gout", tag="gout")
                            sig = nc.scalar.activation(gout[:, :], pga[:, :], AF.Sigmoid)
                            last_sig_inst = sig.ins
                            nc.sync.dma_start(out=gdram.ap()[b, off], in_=gout[:, :])

                # q^T, k^T: [d, tok] per head chunk; stationary = weight tile
                q_sb = [qkp.tile([128, c.S], BF16, name=f"q{h}_sb", tag=f"q{h}")
                        for h in range(NQC)]
                k_sb = [qkp.tile([128, c.S], BF16, name=f"k{h}_sb", tag=f"k{h}")
                        for h in range(NQC)]
                for dst, w in ((q_sb, wq_sb), (k_sb, wk_sb)):
                    for h in range(NQC):
                        for qt in range(c.QTN):
                            psum = pp.tile([128, c.QT], F32, name="psum_qk", tag="pp")
                            for kt in range(c.KTILES):
                                nc.tensor.matmul(
                                    psum[:, :],
                                    w[:, kt, h * 128:(h + 1) * 128],
                                    xt_sb[:, kt, qt * c.QT:(qt + 1) * c.QT],
                                    start=(kt == 0), stop=(kt == c.KTILES - 1))
                            nc.scalar.copy(dst[h][:, qt * c.QT:(qt + 1) * c.QT], psum[:, :])

                # v: [tok, d] natural; stationary = x^T tile
                v_sb = vp.tile([128, c.S // 128, c.DH], BF16, name="v_sb", tag="v")
                for tt in range(c.S // 128):
                    psum = pp.tile([128, c.DH], F32, name="psum_v", tag="pp")
                    for kt in range(c.KTILES):
                        nc.tensor.matmul(
                            psum[:, :],
                            xt_sb[:, kt, tt * 128:(tt + 1) * 128],
                            wv_sb[:, kt, :],
                            start=(kt == 0), stop=(kt == c.KTILES - 1))
                    nc.vector.tensor_copy(v_sb[:, tt, :], psum[:, :])

                # ---- RoPE on q^T / k^T (in place) ----
                # out = t*C2 + swap(t)*S2 ; swap via PE permutation matmul
                for tiles in ((q_sb, k_sb) if use_rope else ()):
                    for h in range(NQC):
                        for qt in range(c.QTN):
                            sl = slice(qt * c.QT, (qt + 1) * c.QT)
                            pswp = pp.tile([128, c.QT], F32, name="pswp", tag="pp")
                            nc.tensor.matmul(pswp[:, :], psw_sb[:, :],
                                             tiles[h][:, sl], start=True, stop=True)
                            m1 = rtp.tile([128, c.QT], BF16, name="rope_m1", tag="m1")
                            m2 = rtp.tile([128, c.QT], BF16, name="rope_m2", tag="m2")
                            nc.vector.tensor_mul(m1[:, :], tiles[h][:, sl], c2_sb[:, sl])
                            nc.vector.tensor_mul(m2[:, :], pswp[:, :], s2_sb[:, sl])
                            nc.vector.tensor_add(tiles[h][:, sl], m1[:, :], m2[:, :])

                # ---- attention (gates + scores + AV + normalize + pout,
                #       interleaved per 512-query block qt) ----
                # normalized per-head outputs og[d, h, tok] stay in SBUF for
                # the output-projection partial matmul
                og_sb = nrm.tile([128, c.HLOC, c.S], BF16, name="og_sb", tag="og")
                for qt in range(c.QTN):
                    qsl = slice(qt * c.QT, (qt + 1) * c.QT)
                    nkt = c.DIAG * qt + c.DIAG  # causal k tiles
                    po = [pop.tile([128, c.QT], F32, name=f"po{h}", tag="po")
                          for h in range(c.HLOC)]
                    prs = [prsp.tile([1, c.QT], F32, name=f"prs{h}", tag="prs")
                           for h in range(c.HLOC)]
                    for kt in range(nkt):
                        ksl = slice(kt * c.KT, (kt + 1) * c.KT)
                        off = (qt * (qt + 1) // 2) * c.DIAG + kt
                        gin = gio.tile([128, c.QT], BF16, name="gin", tag="gin")
                        if use_gate:
                            nc.sync.dma_start(out=gin[:, :], in_=gdram.ap()[b, off])
                        else:
                            nc.vector.memset(gin[:, :], 1.0)
                        for h in range(c.HLOC):
                            ps = psp.tile([128, c.QT], F32, name="ps", tag="ps")
                            nc.tensor.matmul(ps[:, :], k_sb[h][:, ksl],
                                             q_sb[h][:, qsl], start=True, stop=True)
                            p_sb = pge.tile([128, c.QT], BF16, name="p_sb", tag="p")
                            ex = nc.scalar.activation(p_sb[:, :], ps[:, :], AF.Exp,
                                                      scale=isqrt)
                            if use_deps and use_gate and (qt, kt, h) == (0, 0, 0):
                                add_dep_helper(ex.ins, last_sig_inst,
                                               reason="ACT table: exps after this batch's sigmoids")
                            last_exp_inst = ex.ins
                            j = kt - c.DIAG * qt
                            if j >= 0 and use_mask:
                                # diagonal band: causal 0/1 mask applied AFTER
                                # exp (exp(s-1e9)=0 == exp(s)*0); bf16 2x-mode
                                # multiply is cheaper than the fp32 PSUM add
                                pm = pge.tile([128, c.QT], BF16, name="pm", tag="pm")
                                nc.vector.tensor_mul(pm[:, :], p_sb[:, :],
                                                     mask_sb[:, j, :])
                                p_sb = pm
                            # rowsum (pre-gate) via ones-vector matmul
                            if use_rs:
                                nc.tensor.matmul(prs[h][:, :], ones_sb[:, :],
                                                 p_sb[:, :],
                                                 start=(kt == 0), stop=(kt == nkt - 1))
                            pgm = pge.tile([128, c.QT], BF16, name="pgm", tag="pgm")
                            nc.vector.tensor_mul(pgm[:, :], p_sb[:, :], gin[:, :])
                            # out_h^T[d, q] += v[k,d].T @ p_gated[k,q]
                            nc.tensor.matmul(po[h][:, :],
                                             v_sb[:, kt, h * 128:(h + 1) * 128],
                                             pgm[:, :],
                                             start=(kt == 0), stop=(kt == nkt - 1))
                    # normalize: og = po * (1/rowsum) broadcast over partitions
                    for h in range(c.HLOC):
                        ouq = nrm.tile([128, c.QT], F32, name="ouq", tag="ouq")
                        nc.scalar.copy(ouq[:, :], po[h][:, :])
                        rbc = nrm.tile([128, c.QT], F32, name="rbc", tag="rbc")
                        if use_rs and use_bcast:
                            rs = nrm.tile([1, c.QT], F32, name="rs", tag="rs")
                            nc.scalar.copy(rs[:, :], prs[h][:, :])
                            rr = nrm.tile([1, c.QT], F32, name="rr", tag="rr")
                            if use_recip:
                                nc.vector.reciprocal_approx_fast(out=rr[:, :], in_=rs[:, :])
                            else:
                                nc.vector.tensor_copy(rr[:, :], rs[:, :])
                            ridx = (b * c.QTN + qt) * c.HLOC + h
                            rrow = rrd.ap()[ridx:ridx + 1, :]
                            nc.sync.dma_start(out=rrow, in_=rr[:, :])
                            nc.sync.dma_start(
                                out=rbc[:, :],
                                in_=bass.AP(tensor=rrd.ap().tensor, offset=ridx * c.QT,
                                            ap=[[0, 128], [1, c.QT]]))
                        else:
                            nc.vector.memset(rbc[:, :], 1.0)
                        nc.vector.tensor_mul(og_sb[:, h, qsl], ouq[:, :], rbc[:, :])

                    # ---- output-projection partial for this query block:
                    # pout[:, qt] = woc^T.T @ og[:, :, qt] — emitted per qt so
                    # it overlaps the next qt's attention on the PE
                    for ch in range(c.DIM // 128):
                        pf = pp.tile([128, c.QT], F32, name="pf", tag="pp")
                        for h in range(c.HLOC):
                            nc.tensor.matmul(
                                pf[:, :],
                                woc_sb[:, h, ch * 128:(ch + 1) * 128],
                                og_sb[:, h, qsl],
                                start=(h == 0), stop=(h == c.HLOC - 1))
                        f_sb = wop.tile([128, c.QT], F32, name="f_sb", tag="f")
                        nc.vector.tensor_copy(f_sb[:, :], pf[:, :])
                        nc.sync.dma_start(
                            out=pout.ap()[ch * 128:(ch + 1) * 128,
                                          b * c.S + qt * c.QT: b * c.S + (qt + 1) * c.QT],
                            in_=f_sb[:, :])

    nc.compile()
    return nc


def make_core_inputs(inputs: dict, cfg: Cfg = FULL):
    """Host-side sharding: returns in_maps (one dict per core)."""
    c = cfg
    bf16 = ml_dtypes.bfloat16
    x = np.asarray(inputs["x"])
    mask = np.asarray(inputs["mask"])
    fc = np.asarray(inputs["freqs_cos"])
    fs = np.asarray(inputs["freqs_sin"])
    wq, wk, wv, wo = (np.asarray(inputs[k]) for k in ("wq", "wk", "wv", "wo"))
    wa_q, wa_k = np.asarray(inputs["wa_q"]), np.asarray(inputs["wa_k"])

    xT = np.ascontiguousarray(x.transpose(0, 2, 1)).astype(bf16)
    waT = np.ascontiguousarray(np.concatenate([wa_q, wa_k], axis=0).T).astype(bf16)

    # rope tables in [d, tok] layout
    c2 = np.empty((c.HD, c.S), np.float32)
    s2 = np.empty((c.HD, c.S), np.float32)
    c2[0::2] = fc.T
    c2[1::2] = fc.T
    s2[0::2] = -fs.T
    s2[1::2] = fs.T
    c2 = c2.astype(bf16)
    s2 = s2.astype(bf16)

    psw = np.zeros((c.HD, c.HD), np.float32)
    idx = np.arange(c.HD)
    psw[idx, idx ^ 1] = 1.0
    psw = psw.astype(bf16)

    # diagonal-band mask patterns [j][k, q], extracted from the input mask
    qt_last = c.QTN - 1
    q0 = qt_last * c.QT
    maskd = np.empty((c.DIAG, c.KT, c.QT), np.float32)
    for j in range(c.DIAG):
        k0 = (c.DIAG * qt_last + j) * c.KT
        # multiplicative 0/1 form: positions the additive mask leaves at 0
        # (unmasked) become 1.0, masked positions (-1e9) become 0.0
        maskd[j] = (mask[0, 0, q0:q0 + c.QT, k0:k0 + c.KT].T == 0.0)
    maskd = maskd.astype(bf16)

    in_maps = []
    for ci in range(c.NCORES):
        rows = slice(ci * c.DH, (ci + 1) * c.DH)
        in_maps.append({
            "xT": xT,
            "wqT": np.ascontiguousarray(wq[rows].T).astype(bf16),
            "wkT": np.ascontiguousarray(wk[rows].T).astype(bf16),
            "wvT": np.ascontiguousarray(wv[rows].T).astype(bf16),
            "wocT": np.ascontiguousarray(wo[:, rows].T).astype(bf16),
            "waT": waT,
            "c2d": c2,
            "s2d": s2,
            "pswapd": psw,
            "maskdd": maskd,
        })
    return in_maps


def assemble_output(results, cfg: Cfg = FULL) -> np.ndarray:
    c = cfg
    total = np.zeros((c.DIM, c.B * c.S), np.float32)
    for ci in range(c.NCORES):
        total += np.asarray(results[ci]["pout"])
    return np.ascontiguousarray(
        total.reshape(c.DIM, c.B, c.S).transpose(1, 2, 0))


_NC_CACHE = {}


def run(nc, in_maps, trace=False, cfg: Cfg = FULL, **kw):
    return bass_utils.run_bass_kernel_spmd(
        nc, in_maps, core_ids=list(range(cfg.NCORES)), trace=trace, **kw)


def kernel(**inputs) -> np.ndarray:
    cfg = FULL
    if cfg not in _NC_CACHE:
        _NC_CACHE[cfg] = build_nc(cfg)
    nc = _NC_CACHE[cfg]
    in_maps = make_core_inputs(inputs, cfg)
    res = run(nc, in_maps, cfg=cfg)
    return assemble_output(res.results, cfg)


if __name__ == "__main__":
    nc = build_nc(FULL)
    print("built ok")



# revision 10
# speedup vs baseline: 36.6067x; 36.6067x over previous
"""Trainium2 Bass kernel for nn_Attention_10771777978404 (sparse_attention).

Head-parallel (tensor parallel) sharding over 8 NeuronCores:
  - each core owns NH/8 = 2 heads: computes its q/k/v projections (columns of
    wq/wk/wv), RoPE, causal attention with the low-rank sigmoid gate, and the
    per-head attention outputs (transposed, [d, tok]).
  - the rank-32 adapter (gate) weights are replicated; each core computes the
    full [S, S]-gate implicitly, tile by tile, fused into the attention loop.
    The sigmoid is computed as 0.5*(1 + tanh(a/2)) — tanh lives in the SAME
    ACT function table as exp, so gate + softmax exps interleave with zero
    table reloads (sigmoid would force a 1.3us reload per switch). The (1+T)
    is fused into the gate multiply (scalar_tensor_tensor) and the 0.5 into
    the rowsum copy's scale, so the trick costs nothing.
  - rowsum normalization: 1/rowsum broadcast across partitions via the idle
    GpSimd engine (partition_broadcast) instead of a DRAM round-trip.
  - each core emits a full-size PARTIAL of the output projection from its own
    heads (bf16); the host sums the partials across cores (no collective).

Engine queues execute in emission order, so the emitter software-pipelines:
x is streamed per 512-token block, the attention inner loop pre-emits
gate/score matmuls one step ahead of their consumers, and the NEXT batch's
projection work is emitted in small quanta between attention steps so the PE
stays busy during the ACT/DVE-bound attention phase.

Everything on-device is bf16 with fp32 PSUM accumulation.

self-contained: hardcodes the problem shapes; only needs `concourse` (on
PYTHONPATH in this container) + jax axon devices.
"""

import math
from dataclasses import dataclass

import numpy as np
import ml_dtypes

import concourse.bass as bass
import concourse.tile as tile
from concourse import bacc
from concourse import mybir
from concourse import bass_utils

BF16 = mybir.dt.bfloat16
F32 = mybir.dt.float32
AF = mybir.ActivationFunctionType
ALU = mybir.AluOpType


@dataclass(frozen=True)
class Cfg:
    B: int = 2
    S: int = 2048
    DIM: int = 2048
    NH: int = 16
    HD: int = 128
    RANK: int = 32
    NCORES: int = 8
    QT: int = 512   # query block (free dim of score tiles)
    KT: int = 128   # key block (partition dim of score tiles)

    @property
    def HLOC(self):
        return self.NH // self.NCORES

    @property
    def DH(self):
        return self.HLOC * self.HD  # per-core head-dim span

    @property
    def KTILES(self):
        return self.DIM // 128  # contraction tiles for projections

    @property
    def QTN(self):
        return self.S // self.QT

    @property
    def DIAG(self):
        return self.QT // self.KT  # k-tiles per diagonal band


FULL = Cfg()


def build_nc(cfg: Cfg = FULL, *, repeats=1, use_gate=True, use_rs=True,
             use_recip=True, use_rope=True, use_mask=True, wo_act_frac=0.25,
             lookahead=1):
    c = cfg
    assert c.HD == 128 and c.KT == 128
    nc = bacc.Bacc("TRN2", target_bir_lowering=False, debug=False,
                   num_devices=c.NCORES)

    # ---- kernel I/O ----
    xT = nc.dram_tensor("xT", [c.B, c.DIM, c.S], BF16, kind="ExternalInput")
    wqT = nc.dram_tensor("wqT", [c.DIM, c.DH], BF16, kind="ExternalInput")
    wkT = nc.dram_tensor("wkT", [c.DIM, c.DH], BF16, kind="ExternalInput")
    wvT = nc.dram_tensor("wvT", [c.DIM, c.DH], BF16, kind="ExternalInput")
    # woc^T[d_local, j]: this core's head-rows of wo^T (= wo column slice)
    wocT = nc.dram_tensor("wocT", [c.DH, c.DIM], BF16, kind="ExternalInput")
    waT = nc.dram_tensor("waT", [c.DIM, 2 * c.RANK], BF16, kind="ExternalInput")
    c2d = nc.dram_tensor("c2d", [c.HD, c.S], BF16, kind="ExternalInput")
    s2d = nc.dram_tensor("s2d", [c.HD, c.S], BF16, kind="ExternalInput")
    pswapd = nc.dram_tensor("pswapd", [c.HD, c.HD], BF16, kind="ExternalInput")
    maskdd = nc.dram_tensor("maskdd", [c.DIAG, c.KT, c.QT], BF16, kind="ExternalInput")

    # partial output projection, transposed: pout[j, b*S + t] (bf16 partials,
    # summed in f32 on the host)
    pout = nc.dram_tensor("pout", [c.DIM, c.B * c.S], BF16, kind="ExternalOutput")

    isqrt = 1.0 / math.sqrt(c.HD)
    NQC = c.DH // 128          # per-core q/k head chunks (= HLOC)
    NCH = c.DIM // 128         # output column chunks
    NTT = c.QT // 128          # token blocks per q block

    from contextlib import ExitStack
    with ExitStack() as _ctx:
        tc = _ctx.enter_context(tile.TileContext(nc))
        cst = _ctx.enter_context(tc.tile_pool(name="const", bufs=1))
        xtp = _ctx.enter_context(tc.tile_pool(name="xt", bufs=2))
        qkp = _ctx.enter_context(tc.tile_pool(name="qk", bufs=2))
        vp = _ctx.enter_context(tc.tile_pool(name="vp", bufs=2))
        adp = _ctx.enter_context(tc.tile_pool(name="ap", bufs=2))
        rtp = _ctx.enter_context(tc.tile_pool(name="rope_t", bufs=1))
        gio = _ctx.enter_context(tc.tile_pool(name="gio", bufs=3))
        pge = _ctx.enter_context(tc.tile_pool(name="pge", bufs=10))
        nrm = _ctx.enter_context(tc.tile_pool(name="norm", bufs=1))
        ogp = _ctx.enter_context(tc.tile_pool(name="ogp", bufs=1))
        wop = _ctx.enter_context(tc.tile_pool(name="wo_out", bufs=2))
        pp = _ctx.enter_context(tc.tile_pool(name="pp", bufs=2, space="PSUM"))
        psp = _ctx.enter_context(tc.tile_pool(name="ps", bufs=3, space="PSUM"))
        pop = _ctx.enter_context(tc.tile_pool(name="po", bufs=2, space="PSUM"))
        prsp = _ctx.enter_context(tc.tile_pool(name="prs", bufs=1, space="PSUM"))

        # ---- constants / weights (loaded once; reps reuse) ----
        wq_sb = cst.tile([128, c.KTILES, c.DH], BF16, name="wq_sb")
        wk_sb = cst.tile([128, c.KTILES, c.DH], BF16, name="wk_sb")
        wv_sb = cst.tile([128, c.KTILES, c.DH], BF16, name="wv_sb")
        woc_sb = cst.tile([128, NQC, c.DIM], BF16, name="woc_sb")
        wa_sb = cst.tile([128, c.KTILES, 2 * c.RANK], BF16, name="wa_sb")
        c2_sb = cst.tile([128, c.S], BF16, name="c2_sb")
        s2_sb = cst.tile([128, c.S], BF16, name="s2_sb")
        psw_sb = cst.tile([128, 128], BF16, name="psw_sb")
        mask_sb = cst.tile([128, c.DIAG, c.QT], BF16, name="mask_sb")
        ones_sb = cst.tile([128, 1], BF16, name="ones_sb")

        for w_sb, w_d in ((wq_sb, wqT), (wk_sb, wkT), (wv_sb, wvT)):
            wr = w_d.ap().rearrange("(t p) m -> p t m", p=128)
            for half in range(2):
                h0 = half * (c.KTILES // 2)
                nc.sync.dma_start(out=w_sb[:, h0:h0 + c.KTILES // 2, :],
                                  in_=wr[:, h0:h0 + c.KTILES // 2, :])
        wcr = wocT.ap().rearrange("(h p) j -> p h j", p=128)
        for h in range(NQC):
            nc.sync.dma_start(out=woc_sb[:, h, :], in_=wcr[:, h, :])
        nc.sync.dma_start(out=wa_sb, in_=waT.ap().rearrange("(t p) m -> p t m", p=128))
        nc.sync.dma_start(out=c2_sb, in_=c2d.ap())
        nc.sync.dma_start(out=s2_sb, in_=s2d.ap())
        nc.sync.dma_start(out=psw_sb, in_=pswapd.ap())
        nc.sync.dma_start(out=mask_sb, in_=maskdd.ap().rearrange("j p q -> p j q"))
        nc.vector.memset(ones_sb, 1.0)

        def build_proj_ops(b):
            """Projection work for batch b as a list of emit-closures (each
            ~1-2us of PE work). Returns (ops, state)."""
            st = {}
            ops = []

            def alloc():
                st['aqk'] = adp.tile([64, c.S], BF16, name="aqk_sb", tag="aqk")
                st['akl'] = adp.tile([c.RANK, c.S], BF16, name="akl_sb", tag="akl")
                st['q'] = [qkp.tile([128, c.S], BF16, name=f"q{h}_sb", tag=f"q{h}")
                           for h in range(NQC)]
                st['k'] = [qkp.tile([128, c.S], BF16, name=f"k{h}_sb", tag=f"k{h}")
                           for h in range(NQC)]
                st['v'] = vp.tile([128, c.S // 128, c.DH], BF16, name="v_sb", tag="v")
            ops.append(alloc)

            xq = {}

            def load_x(qt):
                def op():
                    t = xtp.tile([128, c.KTILES, c.QT], BF16, name="xtq", tag="xtq")
                    xr = xT.ap()[b].rearrange("(t p) n -> p t n", p=128)
                    nc.sync.dma_start(
                        out=t, in_=xr[:, :, qt * c.QT:(qt + 1) * c.QT])
                    xq[qt] = t
                return op

            def aqk_chain(qt):
                def op():
                    psum = pp.tile([128, c.QT], F32, name="psum_a", tag="pp")
                    for kt in range(c.KTILES):
                        nc.tensor.matmul(psum[0:64, :], wa_sb[:, kt, 0:64],
                                         xq[qt][:, kt, :],
                                         start=(kt == 0), stop=(kt == c.KTILES - 1))
                    nc.vector.tensor_copy(
                        st['aqk'][:, qt * c.QT:(qt + 1) * c.QT], psum[0:64, :])
                return op

            def qk_chain(dst_key, w, h, qt):
                def op():
                    psum = pp.tile([128, c.QT], F32, name="psum_qk", tag="pp")
                    for kt in range(c.KTILES):
                        nc.tensor.matmul(psum[:, :], w[:, kt, h * 128:(h + 1) * 128],
                                         xq[qt][:, kt, :],
                                         start=(kt == 0), stop=(kt == c.KTILES - 1))
                    nc.scalar.copy(
                        st[dst_key][h][:, qt * c.QT:(qt + 1) * c.QT], psum[:, :])
                return op

            def v_chain(qt, tt):
                def op():
                    psum = pp.tile([128, c.QT], F32, name="psum_v", tag="pp")
                    for kt in range(c.KTILES):
                        nc.tensor.matmul(
                            psum[:, 0:c.DH],
                            xq[qt][:, kt, tt * 128:(tt + 1) * 128],
                            wv_sb[:, kt, :],
                            start=(kt == 0), stop=(kt == c.KTILES - 1))
                    nc.vector.tensor_copy(st['v'][:, qt * NTT + tt, :],
                                          psum[:, 0:c.DH])
                return op

            def rope(dst_key, h, qt):
                # out = t*C2 + swap(t)*S2 ; swap via PE permutation matmul.
                # m1 and the final add run on the idle GpSimd engine.
                def op():
                    tl = st[dst_key][h]
                    sl = slice(qt * c.QT, (qt + 1) * c.QT)
                    pswp = pp.tile([128, c.QT], F32, name="pswp", tag="pp")
                    nc.tensor.matmul(pswp[:, :], psw_sb[:, :], tl[:, sl],
                                     start=True, stop=True)
                    m1 = rtp.tile([128, c.QT], BF16, name="rope_m1", tag="m1")
                    m2 = rtp.tile([128, c.QT], BF16, name="rope_m2", tag="m2")
                    nc.gpsimd.tensor_mul(m1[:, :], tl[:, sl], c2_sb[:, sl])
                    nc.vector.tensor_mul(m2[:, :], pswp[:, :], s2_sb[:, sl])
                    nc.gpsimd.tensor_add(tl[:, sl], m1[:, :], m2[:, :])
                return op

            for qt in range(c.QTN):
                ops.append(load_x(qt))
                ops.append(aqk_chain(qt))
                for h in range(NQC):
                    ops.append(qk_chain('q', wq_sb, h, qt))
                    if use_rope:
                        ops.append(rope('q', h, qt))
                for h in range(NQC):
                    ops.append(qk_chain('k', wk_sb, h, qt))
                    if use_rope:
                        ops.append(rope('k', h, qt))
                for tt in range(NTT):
                    ops.append(v_chain(qt, tt))

            def ak_relocate():
                # gate matmul needs ak at base partition 0 (stationary and
                # moving must share a base partition with aq)
                nc.sync.dma_start(out=st['akl'][:, :],
                                  in_=st['aqk'][c.RANK:2 * c.RANK, :])
            ops.append(ak_relocate)
            return ops, st

        def emit_attention(b, st, filler):
            def fill(n=1):
                for _ in range(n):
                    op = next(filler, None)
                    if op is None:
                        return
                    op()

            aq_sb = st['aqk'][0:c.RANK, :]
            ak_sb = st['akl']
            q_sb, k_sb, v_sb = st['q'], st['k'], st['v']
            og_sb = ogp.tile([128, c.HLOC, c.S], BF16, name="og_sb", tag="og")
            wo_acc = 0.0
            for qt in range(c.QTN):
                qsl = slice(qt * c.QT, (qt + 1) * c.QT)
                nkt = c.DIAG * qt + c.DIAG  # causal k tiles
                po = [pop.tile([128, c.QT], F32, name=f"po{h}", tag="po")
                      for h in range(c.HLOC)]
                # both heads' rowsums share one PSUM bank (partitions 0 / 32)
                prs = prsp.tile([33, c.QT], F32, name="prs", tag="prs")
                stash = {}

                def pre(kt):
                    ksl = slice(kt * c.KT, (kt + 1) * c.KT)
                    gt = None
                    if use_gate:
                        # gate tile: T = tanh(a/2); sigmoid(a) = (1+T)/2.
                        # tanh shares the exp ACT table -> no reloads.
                        pga = psp.tile([128, c.QT], F32, name="pga", tag="ps")
                        nc.tensor.matmul(pga[:, :], ak_sb[:, ksl], aq_sb[:, qsl],
                                         start=True, stop=True)
                        gt = gio.tile([128, c.QT], BF16, name="gt", tag="gt")
                        nc.scalar.activation(gt[:, :], pga[:, :], AF.Tanh,
                                             scale=0.5)
                    ptiles = []
                    for h in range(c.HLOC):
                        ps = psp.tile([128, c.QT], F32, name="ps", tag="ps")
                        nc.tensor.matmul(ps[:, :], k_sb[h][:, ksl],
                                         q_sb[h][:, qsl], start=True, stop=True)
                        p_sb = pge.tile([128, c.QT], BF16, name="p_sb", tag="p")
                        nc.scalar.activation(p_sb[:, :], ps[:, :], AF.Exp,
                                             scale=isqrt)
                        j = kt - c.DIAG * qt
                        if j >= 0 and use_mask:
                            # diagonal band: causal 0/1 mask applied AFTER exp
                            # (exp(s-1e9)=0 == exp(s)*0), on GpSimd
                            pm = pge.tile([128, c.QT], BF16, name="pm", tag="pm")
                            nc.gpsimd.tensor_mul(pm[:, :], p_sb[:, :],
                                                 mask_sb[:, j, :])
                            p_sb = pm
                        ptiles.append(p_sb)
                    stash[kt] = (gt, ptiles)

                def cons(kt):
                    gt, ptiles = stash.pop(kt)
                    for h in range(c.HLOC):
                        p_sb = ptiles[h]
                        # rowsum (pre-gate) via ones-vector matmul; the gate's
                        # /2 is deferred to the rs copy below
                        if use_rs:
                            nc.tensor.matmul(prs[32 * h:32 * h + 1, :],
                                             ones_sb[:, :], p_sb[:, :],
                                             start=(kt == 0), stop=(kt == nkt - 1))
                        if use_gate:
                            # p * (1+T) fused in one DVE op
                            pgm = pge.tile([128, c.QT], BF16, name="pgm", tag="pgm")
                            nc.vector.scalar_tensor_tensor(
                                pgm[:, :], gt[:, :], 1.0, p_sb[:, :],
                                op0=ALU.add, op1=ALU.mult)
                        else:
                            pgm = p_sb
                        # out_h^T[d, q] += v[k,d].T @ p_gated[k,q]
                        nc.tensor.matmul(po[h][:, :],
                                         v_sb[:, kt, h * 128:(h + 1) * 128],
                                         pgm[:, :],
                                         start=(kt == 0), stop=(kt == nkt - 1))

                LA = max(1, lookahead)
                for step in range(nkt + LA):
                    if step < nkt:
                        pre(step)
                    if step >= LA:
                        cons(step - LA)
                    fill()

                # normalize: og = po * (1/rowsum); rowsum scaled by 1/2 to
                # absorb the (1+T) = 2*sigmoid factor. Broadcast across
                # partitions on the idle GpSimd engine.
                for h in range(c.HLOC):
                    rs = nrm.tile([1, c.QT], F32, name="rs", tag="rs")
                    nc.scalar.mul(rs[:, :], prs[32 * h:32 * h + 1, :],
                                  2.0 if use_gate else 1.0)
                    rr = nrm.tile([1, c.QT], F32, name="rr", tag="rr")
                    if use_recip:
                        nc.vector.reciprocal_approx_fast(out=rr[:, :], in_=rs[:, :])
                    else:
                        nc.vector.tensor_copy(rr[:, :], rs[:, :])
                    rbc = nrm.tile([128, c.QT], F32, name="rbc", tag="rbc")
                    nc.gpsimd.partition_broadcast(rbc[:, :], rr[:, :])
                    nc.vector.tensor_mul(og_sb[:, h, qsl], po[h][:, :], rbc[:, :])
                    fill()

                # ---- output-projection partial for this query block:
                # pout[:, qt] = woc^T.T @ og[:, :, qt]; column chunks stage
                # into bf16 half-tiles -> 2 DMAs per query block.
                for half in range(2):
                    fq = wop.tile([128, NCH // 2, c.QT], BF16, name="fq", tag="fq")
                    for chh in range(NCH // 2):
                        ch = half * (NCH // 2) + chh
                        pf = pp.tile([128, c.QT], F32, name="pf", tag="pp")
                        for h in range(c.HLOC):
                            nc.tensor.matmul(
                                pf[:, :],
                                woc_sb[:, h, ch * 128:(ch + 1) * 128],
                                og_sb[:, h, qsl],
                                start=(h == 0), stop=(h == c.HLOC - 1))
                        # PSUM->SBUF bf16 copies split between ACT and DVE
                        wo_acc += wo_act_frac
                        if wo_acc >= 1.0:
                            wo_acc -= 1.0
                            nc.scalar.copy(fq[:, chh, :], pf[:, :])
                        else:
                            nc.vector.tensor_copy(fq[:, chh, :], pf[:, :])
                        fill()
                    pr = pout.ap().rearrange("(t p) m -> p t m", p=128)
                    nc.sync.dma_start(
                        out=pr[:, half * (NCH // 2):(half + 1) * (NCH // 2),
                               b * c.S + qt * c.QT: b * c.S + (qt + 1) * c.QT],
                        in_=fq[:, :, :])

        pend = None
        for rep in range(repeats):
            for b in range(c.B):
                ops, st = build_proj_ops(b)
                it = iter(ops)
                if pend is None:
                    for op in it:
                        op()
                else:
                    emit_attention(pend[0], pend[1], it)
                    for op in it:  # leftovers
                        op()
                pend = (b, st)
        emit_attention(pend[0], pend[1], iter(()))

    nc.compile()
    return nc


def make_core_inputs(inputs: dict, cfg: Cfg = FULL):
    """Host-side sharding: returns in_maps (one dict per core)."""
    c = cfg
    bf16 = ml_dtypes.bfloat16
    x = np.asarray(inputs["x"])
    mask = np.asarray(inputs["mask"])
    fc = np.asarray(inputs["freqs_cos"])
    fs = np.asarray(inputs["freqs_sin"])
    wq, wk, wv, wo = (np.asarray(inputs[k]) for k in ("wq", "wk", "wv", "wo"))
    wa_q, wa_k = np.asarray(inputs["wa_q"]), np.asarray(inputs["wa_k"])

    xT = np.ascontiguousarray(x.transpose(0, 2, 1)).astype(bf16)
    waT = np.ascontiguousarray(np.concatenate([wa_q, wa_k], axis=0).T).astype(bf16)

    # rope tables in [d, tok] layout
    c2 = np.empty((c.HD, c.S), np.float32)
    s2 = np.empty((c.HD, c.S), np.float32)
    c2[0::2] = fc.T
    c2[1::2] = fc.T
    s2[0::2] = -fs.T
    s2[1::2] = fs.T
    c2 = c2.astype(bf16)
    s2 = s2.astype(bf16)

    psw = np.zeros((c.HD, c.HD), np.float32)
    idx = np.arange(c.HD)
    psw[idx, idx ^ 1] = 1.0
    psw = psw.astype(bf16)

    # diagonal-band mask patterns [j][k, q], extracted from the input mask
    qt_last = c.QTN - 1
    q0 = qt_last * c.QT
    maskd = np.empty((c.DIAG, c.KT, c.QT), np.float32)
    for j in range(c.DIAG):
        k0 = (c.DIAG * qt_last + j) * c.KT
        # multiplicative 0/1 form: positions the additive mask leaves at 0
        # (unmasked) become 1.0, masked positions (-1e9) become 0.0
        maskd[j] = (mask[0, 0, q0:q0 + c.QT, k0:k0 + c.KT].T == 0.0)
    maskd = maskd.astype(bf16)

    in_maps = []
    for ci in range(c.NCORES):
        rows = slice(ci * c.DH, (ci + 1) * c.DH)
        in_maps.append({
            "xT": xT,
            "wqT": np.ascontiguousarray(wq[rows].T).astype(bf16),
            "wkT": np.ascontiguousarray(wk[rows].T).astype(bf16),
            "wvT": np.ascontiguousarray(wv[rows].T).astype(bf16),
            "wocT": np.ascontiguousarray(wo[:, rows].T).astype(bf16),
            "waT": waT,
            "c2d": c2,
            "s2d": s2,
            "pswapd": psw,
            "maskdd": maskd,
        })
    return in_maps


def assemble_output(results, cfg: Cfg = FULL) -> np.ndarray:
    c = cfg
    total = np.zeros((c.DIM, c.B * c.S), np.float32)
    for ci in range(c.NCORES):
        total += np.asarray(results[ci]["pout"]).astype(np.float32)
    return np.ascontiguousarray(
        total.reshape(c.DIM, c.B, c.S).transpose(1, 2, 0))


_NC_CACHE = {}


def run(nc, in_maps, trace=False, cfg: Cfg = FULL, **kw):
    return bass_utils.run_bass_kernel_spmd(
        nc, in_maps, core_ids=list(range(cfg.NCORES)), trace=trace, **kw)


def kernel(**inputs) -> np.ndarray:
    cfg = FULL
    if cfg not in _NC_CACHE:
        _NC_CACHE[cfg] = build_nc(cfg)
    nc = _NC_CACHE[cfg]
    in_maps = make_core_inputs(inputs, cfg)
    res = run(nc, in_maps, cfg=cfg)
    return assemble_output(res.results, cfg)


if __name__ == "__main__":
    nc = build_nc(FULL)
    print("built ok")


# revision 25
# speedup vs baseline: 116.9702x; 3.1953x over previous
"""Trainium2 Bass kernel for nn_Attention_10771777978404 (sparse_attention).

Head-parallel (tensor parallel) sharding over 8 NeuronCores:
  - each core owns NH/8 = 2 heads: computes its q/k/v projections (columns of
    wq/wk/wv), RoPE, causal attention with the low-rank sigmoid gate, and the
    per-head attention outputs (transposed, [d, tok]).
  - the rank-32 adapter (gate) weights are replicated; each core computes the
    full [S, S]-gate implicitly, tile by tile, fused into the attention loop.
    The sigmoid is computed as 0.5*(1 + tanh(a/2)) — tanh lives in the SAME
    ACT function table as exp, so gate + softmax exps interleave with zero
    table reloads (sigmoid would force a 1.3us reload per switch). The (1+T)
    is fused into the gate multiply (scalar_tensor_tensor) and the 0.5 into
    the rowsum copy's scale, so the trick costs nothing.
  - rowsum normalization: 1/rowsum broadcast across partitions via the idle
    GpSimd engine (partition_broadcast) instead of a DRAM round-trip.
  - each core emits a full-size PARTIAL of the output projection from its own
    heads (bf16); the host sums the partials across cores (no collective).

Engine queues execute in emission order, so the emitter software-pipelines:
x is streamed per 512-token block, the attention inner loop pre-emits
gate/score matmuls one step ahead of their consumers, and the NEXT batch's
projection work is emitted in small quanta between attention steps so the PE
stays busy during the ACT/DVE-bound attention phase.

Everything on-device is bf16 with fp32 PSUM accumulation.

self-contained: hardcodes the problem shapes; only needs `concourse` (on
PYTHONPATH in this container) + jax axon devices.
"""

import math
from dataclasses import dataclass

import numpy as np
import ml_dtypes

import concourse.bass as bass
import concourse.tile as tile
from concourse import bacc
from concourse import mybir
from concourse import bass_utils

BF16 = mybir.dt.bfloat16
F32 = mybir.dt.float32
AF = mybir.ActivationFunctionType
ALU = mybir.AluOpType


@dataclass(frozen=True)
class Cfg:
    B: int = 2
    S: int = 2048
    DIM: int = 2048
    NH: int = 16
    HD: int = 128
    RANK: int = 32
    NCORES: int = 8
    QT: int = 512   # query block (free dim of score tiles)
    KT: int = 128   # key block (partition dim of score tiles)

    @property
    def HLOC(self):
        return self.NH // self.NCORES

    @property
    def DH(self):
        return self.HLOC * self.HD  # per-core head-dim span

    @property
    def KTILES(self):
        return self.DIM // 128  # contraction tiles for projections

    @property
    def QTN(self):
        return self.S // self.QT

    @property
    def DIAG(self):
        return self.QT // self.KT  # k-tiles per diagonal band


FULL = Cfg()


def build_nc(cfg: Cfg = FULL, *, repeats=1, use_gate=True, use_rs=True,
             use_recip=True, use_rope=True, use_mask=True, wo_act_frac=0.25,
             lookahead=1, mask_gp=False, rope_gp=True, interleave=True,
             pair_qk=False):
    c = cfg
    assert c.HD == 128 and c.KT == 128
    nc = bacc.Bacc("TRN2", target_bir_lowering=False, debug=False,
                   num_devices=c.NCORES)

    # ---- kernel I/O ----
    xT = nc.dram_tensor("xT", [c.B, c.DIM, c.S], BF16, kind="ExternalInput")
    wqT = nc.dram_tensor("wqT", [c.DIM, c.DH], BF16, kind="ExternalInput")
    wkT = nc.dram_tensor("wkT", [c.DIM, c.DH], BF16, kind="ExternalInput")
    wvT = nc.dram_tensor("wvT", [c.DIM, c.DH], BF16, kind="ExternalInput")
    # woc^T[d_local, j]: this core's head-rows of wo^T (= wo column slice)
    wocT = nc.dram_tensor("wocT", [c.DH, c.DIM], BF16, kind="ExternalInput")
    waT = nc.dram_tensor("waT", [c.DIM, 2 * c.RANK], BF16, kind="ExternalInput")
    c2d = nc.dram_tensor("c2d", [c.HD, c.S], BF16, kind="ExternalInput")
    s2d = nc.dram_tensor("s2d", [c.HD, c.S], BF16, kind="ExternalInput")
    pswapd = nc.dram_tensor("pswapd", [c.HD, c.HD], BF16, kind="ExternalInput")
    maskdd = nc.dram_tensor("maskdd", [c.DIAG, c.KT, c.QT], BF16, kind="ExternalInput")

    # partial output projection, transposed: pout[j, b*S + t] (bf16 partials,
    # summed in f32 on the host)
    pout = nc.dram_tensor("pout", [c.DIM, c.B * c.S], BF16, kind="ExternalOutput")

    isqrt = 1.0 / math.sqrt(c.HD)
    NQC = c.DH // 128          # per-core q/k head chunks (= HLOC)
    NCH = c.DIM // 128         # output column chunks
    NTT = c.QT // 128          # token blocks per q block

    from contextlib import ExitStack
    with ExitStack() as _ctx:
        tc = _ctx.enter_context(tile.TileContext(nc))
        cst = _ctx.enter_context(tc.tile_pool(name="const", bufs=1))
        xtp = _ctx.enter_context(tc.tile_pool(name="xt", bufs=2))
        qkp = _ctx.enter_context(tc.tile_pool(name="qk", bufs=2))
        vp = _ctx.enter_context(tc.tile_pool(name="vp", bufs=2))
        adp = _ctx.enter_context(tc.tile_pool(name="ap", bufs=2))
        rtp = _ctx.enter_context(tc.tile_pool(name="rope_t", bufs=1))
        gio = _ctx.enter_context(tc.tile_pool(name="gio", bufs=3))
        pge = _ctx.enter_context(tc.tile_pool(name="pge", bufs=10))
        nrm = _ctx.enter_context(tc.tile_pool(name="norm", bufs=1))
        ogp = _ctx.enter_context(tc.tile_pool(name="ogp", bufs=1))
        wop = _ctx.enter_context(tc.tile_pool(name="wo_out", bufs=2))
        pp = _ctx.enter_context(tc.tile_pool(name="pp", bufs=2, space="PSUM"))
        psp = _ctx.enter_context(tc.tile_pool(name="ps", bufs=3, space="PSUM"))
        pop = _ctx.enter_context(tc.tile_pool(name="po", bufs=2, space="PSUM"))
        prsp = _ctx.enter_context(tc.tile_pool(name="prs", bufs=1, space="PSUM"))

        # ---- constants / weights (loaded once; reps reuse) ----
        wq_sb = cst.tile([128, c.KTILES, c.DH], BF16, name="wq_sb")
        wk_sb = cst.tile([128, c.KTILES, c.DH], BF16, name="wk_sb")
        wv_sb = cst.tile([128, c.KTILES, c.DH], BF16, name="wv_sb")
        woc_sb = cst.tile([128, NQC, c.DIM], BF16, name="woc_sb")
        wa_sb = cst.tile([128, c.KTILES, 2 * c.RANK], BF16, name="wa_sb")
        c2_sb = cst.tile([128, c.S], BF16, name="c2_sb")
        s2_sb = cst.tile([128, c.S], BF16, name="s2_sb")
        psw_sb = cst.tile([128, 128], BF16, name="psw_sb")
        mask_sb = cst.tile([128, c.DIAG, c.QT], BF16, name="mask_sb")
        ones_sb = cst.tile([128, 64], BF16, name="ones_sb")

        for w_sb, w_d in ((wq_sb, wqT), (wk_sb, wkT), (wv_sb, wvT)):
            wr = w_d.ap().rearrange("(t p) m -> p t m", p=128)
            for half in range(2):
                h0 = half * (c.KTILES // 2)
                nc.sync.dma_start(out=w_sb[:, h0:h0 + c.KTILES // 2, :],
                                  in_=wr[:, h0:h0 + c.KTILES // 2, :])
        wcr = wocT.ap().rearrange("(h p) j -> p h j", p=128)
        for h in range(NQC):
            nc.sync.dma_start(out=woc_sb[:, h, :], in_=wcr[:, h, :])
        nc.sync.dma_start(out=wa_sb, in_=waT.ap().rearrange("(t p) m -> p t m", p=128))
        nc.sync.dma_start(out=c2_sb, in_=c2d.ap())
        nc.sync.dma_start(out=s2_sb, in_=s2d.ap())
        nc.sync.dma_start(out=psw_sb, in_=pswapd.ap())
        nc.sync.dma_start(out=mask_sb, in_=maskdd.ap().rearrange("j p q -> p j q"))
        nc.vector.memset(ones_sb, 1.0)

        def build_proj_ops(b):
            """Projection work for batch b as a list of emit-closures (each
            ~1-2us of PE work). Returns (ops, state)."""
            st = {}
            ops = []

            def alloc():
                st['aqk'] = adp.tile([64, c.S], BF16, name="aqk_sb", tag="aqk")
                st['akl'] = adp.tile([c.RANK, c.S], BF16, name="akl_sb", tag="akl")
                st['q'] = [qkp.tile([128, c.S], BF16, name=f"q{h}_sb", tag=f"q{h}")
                           for h in range(NQC)]
                st['k'] = [qkp.tile([128, c.S], BF16, name=f"k{h}_sb", tag=f"k{h}")
                           for h in range(NQC)]
                st['v'] = vp.tile([128, c.S // 128, c.DH], BF16, name="v_sb", tag="v")
            ops.append((0.1, alloc))

            xq = {}

            def load_x(qt):
                def op():
                    t = xtp.tile([128, c.KTILES, c.QT], BF16, name="xtq", tag="xtq")
                    xr = xT.ap()[b].rearrange("(t p) n -> p t n", p=128)
                    nc.sync.dma_start(
                        out=t, in_=xr[:, :, qt * c.QT:(qt + 1) * c.QT])
                    xq[qt] = t
                return op

            def aqk_chain(qt, n):
                # n query blocks per chain: one stationary load feeds all
                def op():
                    psums = [pp.tile([128, c.QT], F32, name="psum_a", tag="pp")
                             for _ in range(n)]
                    for kt in range(c.KTILES):
                        for i in range(n):
                            nc.tensor.matmul(
                                psums[i][0:64, :], wa_sb[:, kt, 0:64],
                                xq[qt + i][:, kt, :],
                                start=(kt == 0), stop=(kt == c.KTILES - 1))
                    for i in range(n):
                        nc.vector.tensor_copy(
                            st['aqk'][:, (qt + i) * c.QT:(qt + i + 1) * c.QT],
                            psums[i][0:64, :])
                return op

            def qk_chain(dst_key, w, h, qt, n):
                # n query blocks per chain: one stationary load feeds all
                def op():
                    psums = [pp.tile([128, c.QT], F32, name="psum_qk", tag="pp")
                             for _ in range(n)]
                    for kt in range(c.KTILES):
                        for i in range(n):
                            nc.tensor.matmul(
                                psums[i][:, :], w[:, kt, h * 128:(h + 1) * 128],
                                xq[qt + i][:, kt, :],
                                start=(kt == 0), stop=(kt == c.KTILES - 1))
                    for i in range(n):
                        nc.scalar.copy(
                            st[dst_key][h][:, (qt + i) * c.QT:(qt + i + 1) * c.QT],
                            psums[i][:, :])
                return op

            def v_chain(qt, tt):
                def op():
                    psum = pp.tile([128, c.QT], F32, name="psum_v", tag="pp")
                    for kt in range(c.KTILES):
                        nc.tensor.matmul(
                            psum[:, 0:c.DH],
                            xq[qt][:, kt, tt * 128:(tt + 1) * 128],
                            wv_sb[:, kt, :],
                            start=(kt == 0), stop=(kt == c.KTILES - 1))
                    nc.vector.tensor_copy(st['v'][:, qt * NTT + tt, :],
                                          psum[:, 0:c.DH])
                return op

            def rope(dst_key, h, qt):
                # out = t*C2 + swap(t)*S2 ; swap via PE permutation matmul.
                # m1 and the final add run on the idle GpSimd engine.
                def op():
                    tl = st[dst_key][h]
                    sl = slice(qt * c.QT, (qt + 1) * c.QT)
                    pswp = pp.tile([128, c.QT], F32, name="pswp", tag="pp")
                    nc.tensor.matmul(pswp[:, :], psw_sb[:, :], tl[:, sl],
                                     start=True, stop=True)
                    m1 = rtp.tile([128, c.QT], BF16, name="rope_m1", tag="m1")
                    m2 = rtp.tile([128, c.QT], BF16, name="rope_m2", tag="m2")
                    eng1 = nc.gpsimd if rope_gp else nc.vector
                    eng1.tensor_mul(m1[:, :], tl[:, sl], c2_sb[:, sl])
                    nc.vector.tensor_mul(m2[:, :], pswp[:, :], s2_sb[:, sl])
                    eng1.tensor_add(tl[:, sl], m1[:, :], m2[:, :])
                return op

            PAIR = 2 if pair_qk else 1
            for qp in range(0, c.QTN, PAIR):
                for i in range(PAIR):
                    ops.append((0.3, load_x(qp + i)))
                ops.append((PAIR, aqk_chain(qp, PAIR)))
                for h in range(NQC):
                    ops.append((PAIR, qk_chain('q', wq_sb, h, qp, PAIR)))
                    if use_rope:
                        for i in range(PAIR):
                            ops.append((0.5, rope('q', h, qp + i)))
                for h in range(NQC):
                    ops.append((PAIR, qk_chain('k', wk_sb, h, qp, PAIR)))
                    if use_rope:
                        for i in range(PAIR):
                            ops.append((0.5, rope('k', h, qp + i)))
                for qt in range(qp, qp + PAIR):
                    for tt in range(NTT):
                        ops.append((1, v_chain(qt, tt)))

            def ak_relocate():
                # gate matmul needs ak at base partition 0 (stationary and
                # moving must share a base partition with aq)
                nc.sync.dma_start(out=st['akl'][:, :],
                                  in_=st['aqk'][c.RANK:2 * c.RANK, :])
            ops.append((0.3, ak_relocate))
            return ops, st

        def emit_attention(b, st, filler):
            credit = [0.0]

            def fill(n=1.0):
                # cost-weighted pacing: accumulate credit, pop ops while
                # affordable so big paired chains don't jam the pipeline
                credit[0] += n
                while credit[0] > 0:
                    item = next(filler, None)
                    if item is None:
                        return
                    cost, op = item
                    op()
                    credit[0] -= cost

            aq_sb = st['aqk'][0:c.RANK, :]
            ak_sb = st['akl']
            q_sb, k_sb, v_sb = st['q'], st['k'], st['v']
            og_sb = ogp.tile([128, c.HLOC, c.S], BF16, name="og_sb", tag="og")
            wo_acc = 0.0
            for qt in range(c.QTN):
                qsl = slice(qt * c.QT, (qt + 1) * c.QT)
                nkt = c.DIAG * qt + c.DIAG  # causal k tiles
                po = [pop.tile([128, c.QT], F32, name=f"po{h}", tag="po")
                      for h in range(c.HLOC)]
                # both heads' rowsums share one PSUM bank, at partitions 0
                # and 32 (separate hardware zero regions)
                prs = prsp.tile([33, c.QT], F32, name="prs", tag="prs")
                stash = {}

                def pre(kt):
                    ksl = slice(kt * c.KT, (kt + 1) * c.KT)
                    gt = None
                    if use_gate:
                        # gate tile: T = tanh(a/2); sigmoid(a) = (1+T)/2.
                        # tanh shares the exp ACT table -> no reloads.
                        pga = psp.tile([128, c.QT], F32, name="pga", tag="ps")
                        nc.tensor.matmul(pga[:, :], ak_sb[:, ksl], aq_sb[:, qsl],
                                         start=True, stop=True)
                        gt = gio.tile([128, c.QT], BF16, name="gt", tag="gt")
                        nc.scalar.activation(gt[:, :], pga[:, :], AF.Tanh,
                                             scale=0.5)
                    ptiles = []
                    for h in range(c.HLOC):
                        ps = psp.tile([128, c.QT], F32, name="ps", tag="ps")
                        nc.tensor.matmul(ps[:, :], k_sb[h][:, ksl],
                                         q_sb[h][:, qsl], start=True, stop=True)
                        p_sb = pge.tile([128, c.QT], BF16, name="p_sb", tag="p")
                        nc.scalar.activation(p_sb[:, :], ps[:, :], AF.Exp,
                                             scale=isqrt)
                        j = kt - c.DIAG * qt
                        if j >= 0 and use_mask:
                            # diagonal band: causal 0/1 mask applied AFTER exp
                            # (exp(s-1e9)=0 == exp(s)*0)
                            pm = pge.tile([128, c.QT], BF16, name="pm", tag="pm")
                            (nc.gpsimd if mask_gp else nc.vector).tensor_mul(
                                pm[:, :], p_sb[:, :], mask_sb[:, j, :])
                            p_sb = pm
                        ptiles.append(p_sb)
                    stash[kt] = (gt, ptiles)

                def cons(kt):
                    gt, ptiles = stash.pop(kt)
                    for h in range(c.HLOC):
                        p_sb = ptiles[h]
                        # rowsum (pre-gate) via ones-vector matmul; the gate's
                        # /2 is deferred to the rs copy below
                        if use_rs:
                            nc.tensor.matmul(prs[32 * h:32 * h + 1, :],
                                             ones_sb[:, 0:1], p_sb[:, :],
                                             start=(kt == 0), stop=(kt == nkt - 1))
                        if use_gate:
                            # p * (1+T) fused in one DVE op
                            pgm = pge.tile([128, c.QT], BF16, name="pgm", tag="pgm")
                            nc.vector.scalar_tensor_tensor(
                                pgm[:, :], gt[:, :], 1.0, p_sb[:, :],
                                op0=ALU.add, op1=ALU.mult)
                        else:
                            pgm = p_sb
                        # out_h^T[d, q] += v[k,d].T @ p_gated[k,q]
                        nc.tensor.matmul(po[h][:, :],
                                         v_sb[:, kt, h * 128:(h + 1) * 128],
                                         pgm[:, :],
                                         start=(kt == 0), stop=(kt == nkt - 1))

                LA = max(1, lookahead)
                for step in range(nkt + LA):
                    if step < nkt:
                        pre(step)
                    if step >= LA:
                        cons(step - LA)
                    fill()

                # normalize: og = po * (1/rowsum); rowsum scaled by 1/2 to
                # absorb the (1+T) = 2*sigmoid factor. Broadcast across
                # partitions on the idle GpSimd engine.
                for h in range(c.HLOC):
                    rs = nrm.tile([1, c.QT], F32, name="rs", tag="rs")
                    nc.scalar.mul(rs[:, :], prs[32 * h:32 * h + 1, :],
                                  2.0 if use_gate else 1.0)
                    rr = nrm.tile([1, c.QT], F32, name="rr", tag="rr")
                    if use_recip:
                        nc.vector.reciprocal_approx_fast(out=rr[:, :], in_=rs[:, :])
                    else:
                        nc.vector.tensor_copy(rr[:, :], rs[:, :])
                    rbc = nrm.tile([128, c.QT], F32, name="rbc", tag="rbc")
                    nc.gpsimd.partition_broadcast(rbc[:, :], rr[:, :])
                    nc.vector.tensor_mul(og_sb[:, h, qsl], po[h][:, :], rbc[:, :])
                    fill()

                # ---- output-projection partial for this query block:
                # pout[:, qt] = woc^T.T @ og[:, :, qt]; column chunks stage
                # into bf16 half-tiles -> 2 DMAs per query block.
                for half in range(2):
                    fq = wop.tile([128, NCH // 2, c.QT], BF16, name="fq", tag="fq")
                    for chh in range(NCH // 2):
                        ch = half * (NCH // 2) + chh
                        pf = pp.tile([128, c.QT], F32, name="pf", tag="pp")
                        for h in range(c.HLOC):
                            nc.tensor.matmul(
                                pf[:, :],
                                woc_sb[:, h, ch * 128:(ch + 1) * 128],
                                og_sb[:, h, qsl],
                                start=(h == 0), stop=(h == c.HLOC - 1))
                        # PSUM->SBUF bf16 copies split between ACT and DVE
                        wo_acc += wo_act_frac
                        if wo_acc >= 1.0:
                            wo_acc -= 1.0
                            nc.scalar.copy(fq[:, chh, :], pf[:, :])
                        else:
                            nc.vector.tensor_copy(fq[:, chh, :], pf[:, :])
                        fill()
                    pr = pout.ap().rearrange("(t p) m -> p t m", p=128)
                    nc.sync.dma_start(
                        out=pr[:, half * (NCH // 2):(half + 1) * (NCH // 2),
                               b * c.S + qt * c.QT: b * c.S + (qt + 1) * c.QT],
                        in_=fq[:, :, :])

        pend = None
        for rep in range(repeats):
            for b in range(c.B):
                ops, st = build_proj_ops(b)
                it = iter(ops)
                if pend is None:
                    for _, op in it:
                        op()
                else:
                    emit_attention(pend[0], pend[1],
                                   it if interleave else iter(()))
                    for _, op in it:  # leftovers
                        op()
                pend = (b, st)
        emit_attention(pend[0], pend[1], iter(()))


    nc.compile()
    return nc


def make_core_inputs(inputs: dict, cfg: Cfg = FULL):
    """Host-side sharding: returns in_maps (one dict per core)."""
    c = cfg
    bf16 = ml_dtypes.bfloat16
    x = np.asarray(inputs["x"])
    mask = np.asarray(inputs["mask"])
    fc = np.asarray(inputs["freqs_cos"])
    fs = np.asarray(inputs["freqs_sin"])
    wq, wk, wv, wo = (np.asarray(inputs[k]) for k in ("wq", "wk", "wv", "wo"))
    wa_q, wa_k = np.asarray(inputs["wa_q"]), np.asarray(inputs["wa_k"])

    xT = np.ascontiguousarray(x.transpose(0, 2, 1)).astype(bf16)
    waT = np.ascontiguousarray(np.concatenate([wa_q, wa_k], axis=0).T).astype(bf16)

    # rope tables in [d, tok] layout
    c2 = np.empty((c.HD, c.S), np.float32)
    s2 = np.empty((c.HD, c.S), np.float32)
    c2[0::2] = fc.T
    c2[1::2] = fc.T
    s2[0::2] = -fs.T
    s2[1::2] = fs.T
    c2 = c2.astype(bf16)
    s2 = s2.astype(bf16)

    psw = np.zeros((c.HD, c.HD), np.float32)
    idx = np.arange(c.HD)
    psw[idx, idx ^ 1] = 1.0
    psw = psw.astype(bf16)

    # diagonal-band mask patterns [j][k, q], extracted from the input mask
    qt_last = c.QTN - 1
    q0 = qt_last * c.QT
    maskd = np.empty((c.DIAG, c.KT, c.QT), np.float32)
    for j in range(c.DIAG):
        k0 = (c.DIAG * qt_last + j) * c.KT
        # multiplicative 0/1 form: positions the additive mask leaves at 0
        # (unmasked) become 1.0, masked positions (-1e9) become 0.0
        maskd[j] = (mask[0, 0, q0:q0 + c.QT, k0:k0 + c.KT].T == 0.0)
    maskd = maskd.astype(bf16)

    in_maps = []
    for ci in range(c.NCORES):
        rows = slice(ci * c.DH, (ci + 1) * c.DH)
        in_maps.append({
            "xT": xT,
            "wqT": np.ascontiguousarray(wq[rows].T).astype(bf16),
            "wkT": np.ascontiguousarray(wk[rows].T).astype(bf16),
            "wvT": np.ascontiguousarray(wv[rows].T).astype(bf16),
            "wocT": np.ascontiguousarray(wo[:, rows].T).astype(bf16),
            "waT": waT,
            "c2d": c2,
            "s2d": s2,
            "pswapd": psw,
            "maskdd": maskd,
        })
    return in_maps


def assemble_output(results, cfg: Cfg = FULL) -> np.ndarray:
    c = cfg
    total = np.zeros((c.DIM, c.B * c.S), np.float32)
    for ci in range(c.NCORES):
        total += np.asarray(results[ci]["pout"]).astype(np.float32)
    return np.ascontiguousarray(
        total.reshape(c.DIM, c.B, c.S).transpose(1, 2, 0))


_NC_CACHE = {}


def run(nc, in_maps, trace=False, cfg: Cfg = FULL, **kw):
    return bass_utils.run_bass_kernel_spmd(
        nc, in_maps, core_ids=list(range(cfg.NCORES)), trace=trace, **kw)


def kernel(**inputs) -> np.ndarray:
    cfg = FULL
    if cfg not in _NC_CACHE:
        _NC_CACHE[cfg] = build_nc(cfg)
    nc = _NC_CACHE[cfg]
    in_maps = make_core_inputs(inputs, cfg)
    res = run(nc, in_maps, cfg=cfg)
    return assemble_output(res.results, cfg)


if __name__ == "__main__":
    nc = build_nc(FULL)
    print("built ok")


# revision 26
# speedup vs baseline: 121.4477x; 1.0383x over previous
"""Trainium2 Bass kernel for nn_Attention_10771777978404 (sparse_attention).

Head-parallel (tensor parallel) sharding over 8 NeuronCores:
  - each core owns NH/8 = 2 heads: computes its q/k/v projections (columns of
    wq/wk/wv), RoPE, causal attention with the low-rank sigmoid gate, and the
    per-head attention outputs (transposed, [d, tok]).
  - the rank-32 adapter (gate) weights are replicated; each core computes the
    full [S, S]-gate implicitly, tile by tile, fused into the attention loop.
    The sigmoid is computed as 0.5*(1 + tanh(a/2)) — tanh lives in the SAME
    ACT function table as exp, so gate + softmax exps interleave with zero
    table reloads (sigmoid would force a 1.3us reload per switch). The (1+T)
    is fused into the gate multiply (scalar_tensor_tensor) and the
    compensating 2x into the rowsum copy's scale, so the trick costs nothing.
  - rowsum normalization: 1/rowsum broadcast across partitions via the idle
    GpSimd engine (partition_broadcast) instead of a DRAM round-trip.
  - each core emits a full-size PARTIAL of the output projection from its own
    heads (bf16); the host sums the partials across cores (no collective).

Engine queues execute in emission order, so the emitter software-pipelines:
x is streamed per 512-token block, the attention inner loop pre-emits
gate/score matmuls one step ahead of their consumers, and the NEXT batch's
projection work is emitted in small quanta between attention steps so the PE
stays busy during the ACT/DVE-bound attention phase.

Everything on-device is bf16 with fp32 PSUM accumulation.

self-contained: hardcodes the problem shapes; only needs `concourse` (on
PYTHONPATH in this container) + jax axon devices.
"""

import math
from dataclasses import dataclass

import numpy as np
import ml_dtypes

import concourse.bass as bass
import concourse.tile as tile
from concourse import bacc
from concourse import mybir
from concourse import bass_utils

BF16 = mybir.dt.bfloat16
F32 = mybir.dt.float32
AF = mybir.ActivationFunctionType
ALU = mybir.AluOpType


@dataclass(frozen=True)
class Cfg:
    B: int = 2
    S: int = 2048
    DIM: int = 2048
    NH: int = 16
    HD: int = 128
    RANK: int = 32
    NCORES: int = 8
    QT: int = 512   # query block (free dim of score tiles)
    KT: int = 128   # key block (partition dim of score tiles)

    @property
    def HLOC(self):
        return self.NH // self.NCORES

    @property
    def DH(self):
        return self.HLOC * self.HD  # per-core head-dim span

    @property
    def KTILES(self):
        return self.DIM // 128  # contraction tiles for projections

    @property
    def QTN(self):
        return self.S // self.QT

    @property
    def DIAG(self):
        return self.QT // self.KT  # k-tiles per diagonal band


FULL = Cfg()


def build_nc(cfg: Cfg = FULL, *, repeats=1, use_gate=True, use_rs=True,
             use_recip=True, use_rope=True, use_mask=True, wo_act_frac=0.25,
             lookahead=1, mask_gp=False, rope_gp=True, interleave=True,
             pair_qk=False):
    c = cfg
    assert c.HD == 128 and c.KT == 128
    nc = bacc.Bacc("TRN2", target_bir_lowering=False, debug=False,
                   num_devices=c.NCORES)

    # ---- kernel I/O ----
    xT = nc.dram_tensor("xT", [c.B, c.DIM, c.S], BF16, kind="ExternalInput")
    wqT = nc.dram_tensor("wqT", [c.DIM, c.DH], BF16, kind="ExternalInput")
    wkT = nc.dram_tensor("wkT", [c.DIM, c.DH], BF16, kind="ExternalInput")
    wvT = nc.dram_tensor("wvT", [c.DIM, c.DH], BF16, kind="ExternalInput")
    # woc^T[d_local, j]: this core's head-rows of wo^T (= wo column slice)
    wocT = nc.dram_tensor("wocT", [c.DH, c.DIM], BF16, kind="ExternalInput")
    waT = nc.dram_tensor("waT", [c.DIM, 2 * c.RANK], BF16, kind="ExternalInput")
    c2d = nc.dram_tensor("c2d", [c.HD, c.S], BF16, kind="ExternalInput")
    s2d = nc.dram_tensor("s2d", [c.HD, c.S], BF16, kind="ExternalInput")
    pswapd = nc.dram_tensor("pswapd", [c.HD, c.HD], BF16, kind="ExternalInput")
    maskdd = nc.dram_tensor("maskdd", [c.DIAG, c.KT, c.QT], BF16, kind="ExternalInput")

    # partial output projection, transposed: pout[j, b*S + t] (bf16 partials,
    # summed in f32 on the host)
    pout = nc.dram_tensor("pout", [c.DIM, c.B * c.S], BF16, kind="ExternalOutput")

    isqrt = 1.0 / math.sqrt(c.HD)
    NQC = c.DH // 128          # per-core q/k head chunks (= HLOC)
    NCH = c.DIM // 128         # output column chunks
    NTT = c.QT // 128          # token blocks per q block

    from contextlib import ExitStack
    with ExitStack() as _ctx:
        tc = _ctx.enter_context(tile.TileContext(nc))
        cst = _ctx.enter_context(tc.tile_pool(name="const", bufs=1))
        xtp = _ctx.enter_context(tc.tile_pool(name="xt", bufs=2))
        qkp = _ctx.enter_context(tc.tile_pool(name="qk", bufs=2))
        vp = _ctx.enter_context(tc.tile_pool(name="vp", bufs=2))
        adp = _ctx.enter_context(tc.tile_pool(name="ap", bufs=2))
        rtp = _ctx.enter_context(tc.tile_pool(name="rope_t", bufs=1))
        gio = _ctx.enter_context(tc.tile_pool(name="gio", bufs=3))
        pge = _ctx.enter_context(tc.tile_pool(name="pge", bufs=10))
        nrm = _ctx.enter_context(tc.tile_pool(name="norm", bufs=1))
        ogp = _ctx.enter_context(tc.tile_pool(name="ogp", bufs=1))
        wop = _ctx.enter_context(tc.tile_pool(name="wo_out", bufs=2))
        pp = _ctx.enter_context(tc.tile_pool(name="pp", bufs=2, space="PSUM"))
        psp = _ctx.enter_context(tc.tile_pool(name="ps", bufs=3, space="PSUM"))
        pop = _ctx.enter_context(tc.tile_pool(name="po", bufs=2, space="PSUM"))
        prsp = _ctx.enter_context(tc.tile_pool(name="prs", bufs=1, space="PSUM"))

        # ---- constants / weights (loaded once; reps reuse) ----
        wq_sb = cst.tile([128, c.KTILES, c.DH], BF16, name="wq_sb")
        wk_sb = cst.tile([128, c.KTILES, c.DH], BF16, name="wk_sb")
        wv_sb = cst.tile([128, c.KTILES, c.DH], BF16, name="wv_sb")
        woc_sb = cst.tile([128, NQC, c.DIM], BF16, name="woc_sb")
        wa_sb = cst.tile([128, c.KTILES, 2 * c.RANK], BF16, name="wa_sb")
        c2_sb = cst.tile([128, c.S], BF16, name="c2_sb")
        s2_sb = cst.tile([128, c.S], BF16, name="s2_sb")
        psw_sb = cst.tile([128, 128], BF16, name="psw_sb")
        mask_sb = cst.tile([128, c.DIAG, c.QT], BF16, name="mask_sb")
        ones_sb = cst.tile([128, 64], BF16, name="ones_sb")

        for w_sb, w_d in ((wq_sb, wqT), (wk_sb, wkT), (wv_sb, wvT)):
            wr = w_d.ap().rearrange("(t p) m -> p t m", p=128)
            for half in range(2):
                h0 = half * (c.KTILES // 2)
                nc.sync.dma_start(out=w_sb[:, h0:h0 + c.KTILES // 2, :],
                                  in_=wr[:, h0:h0 + c.KTILES // 2, :])
        wcr = wocT.ap().rearrange("(h p) j -> p h j", p=128)
        for h in range(NQC):
            nc.sync.dma_start(out=woc_sb[:, h, :], in_=wcr[:, h, :])
        nc.sync.dma_start(out=wa_sb, in_=waT.ap().rearrange("(t p) m -> p t m", p=128))
        nc.sync.dma_start(out=c2_sb, in_=c2d.ap())
        nc.sync.dma_start(out=s2_sb, in_=s2d.ap())
        nc.sync.dma_start(out=psw_sb, in_=pswapd.ap())
        nc.sync.dma_start(out=mask_sb, in_=maskdd.ap().rearrange("j p q -> p j q"))
        nc.vector.memset(ones_sb, 1.0)

        def build_proj_ops(b):
            """Projection work for batch b as a list of emit-closures (each
            ~1-2us of PE work). Returns (ops, state)."""
            st = {}
            ops = []

            def alloc():
                st['aqk'] = adp.tile([64, c.S], BF16, name="aqk_sb", tag="aqk")
                st['akl'] = adp.tile([c.RANK, c.S], BF16, name="akl_sb", tag="akl")
                st['q'] = [qkp.tile([128, c.S], BF16, name=f"q{h}_sb", tag=f"q{h}")
                           for h in range(NQC)]
                st['k'] = [qkp.tile([128, c.S], BF16, name=f"k{h}_sb", tag=f"k{h}")
                           for h in range(NQC)]
                st['v'] = vp.tile([128, c.S // 128, c.DH], BF16, name="v_sb", tag="v")
            ops.append((0.1, alloc))

            xq = {}

            def load_x(qt):
                def op():
                    t = xtp.tile([128, c.KTILES, c.QT], BF16, name="xtq", tag="xtq")
                    xr = xT.ap()[b].rearrange("(t p) n -> p t n", p=128)
                    nc.sync.dma_start(
                        out=t, in_=xr[:, :, qt * c.QT:(qt + 1) * c.QT])
                    xq[qt] = t
                return op

            def aqk_chain(qt, n):
                # n query blocks per chain: one stationary load feeds all
                def op():
                    psums = [pp.tile([128, c.QT], F32, name="psum_a", tag="pp")
                             for _ in range(n)]
                    for kt in range(c.KTILES):
                        for i in range(n):
                            nc.tensor.matmul(
                                psums[i][0:64, :], wa_sb[:, kt, 0:64],
                                xq[qt + i][:, kt, :],
                                start=(kt == 0), stop=(kt == c.KTILES - 1))
                    for i in range(n):
                        nc.vector.tensor_copy(
                            st['aqk'][:, (qt + i) * c.QT:(qt + i + 1) * c.QT],
                            psums[i][0:64, :])
                return op

            def qk_chain(dst_key, w, h, qt, n):
                # n query blocks per chain: one stationary load feeds all
                def op():
                    psums = [pp.tile([128, c.QT], F32, name="psum_qk", tag="pp")
                             for _ in range(n)]
                    for kt in range(c.KTILES):
                        for i in range(n):
                            nc.tensor.matmul(
                                psums[i][:, :], w[:, kt, h * 128:(h + 1) * 128],
                                xq[qt + i][:, kt, :],
                                start=(kt == 0), stop=(kt == c.KTILES - 1))
                    for i in range(n):
                        nc.scalar.copy(
                            st[dst_key][h][:, (qt + i) * c.QT:(qt + i + 1) * c.QT],
                            psums[i][:, :])
                return op

            def v_chain(qt, tt):
                def op():
                    psum = pp.tile([128, c.QT], F32, name="psum_v", tag="pp")
                    for kt in range(c.KTILES):
                        nc.tensor.matmul(
                            psum[:, 0:c.DH],
                            xq[qt][:, kt, tt * 128:(tt + 1) * 128],
                            wv_sb[:, kt, :],
                            start=(kt == 0), stop=(kt == c.KTILES - 1))
                    nc.vector.tensor_copy(st['v'][:, qt * NTT + tt, :],
                                          psum[:, 0:c.DH])
                return op

            def rope(dst_key, h, qt):
                # out = t*C2 + swap(t)*S2 ; swap via PE permutation matmul.
                # m1 and the final add run on the idle GpSimd engine.
                def op():
                    tl = st[dst_key][h]
                    sl = slice(qt * c.QT, (qt + 1) * c.QT)
                    pswp = pp.tile([128, c.QT], F32, name="pswp", tag="pp")
                    nc.tensor.matmul(pswp[:, :], psw_sb[:, :], tl[:, sl],
                                     start=True, stop=True)
                    m1 = rtp.tile([128, c.QT], BF16, name="rope_m1", tag="m1")
                    m2 = rtp.tile([128, c.QT], BF16, name="rope_m2", tag="m2")
                    eng1 = nc.gpsimd if rope_gp else nc.vector
                    eng1.tensor_mul(m1[:, :], tl[:, sl], c2_sb[:, sl])
                    nc.vector.tensor_mul(m2[:, :], pswp[:, :], s2_sb[:, sl])
                    eng1.tensor_add(tl[:, sl], m1[:, :], m2[:, :])
                return op

            PAIR = 2 if pair_qk else 1
            for qp in range(0, c.QTN, PAIR):
                for i in range(PAIR):
                    ops.append((0.3, load_x(qp + i)))
                ops.append((PAIR, aqk_chain(qp, PAIR)))
                for h in range(NQC):
                    ops.append((PAIR, qk_chain('q', wq_sb, h, qp, PAIR)))
                    if use_rope:
                        for i in range(PAIR):
                            ops.append((0.5, rope('q', h, qp + i)))
                for h in range(NQC):
                    ops.append((PAIR, qk_chain('k', wk_sb, h, qp, PAIR)))
                    if use_rope:
                        for i in range(PAIR):
                            ops.append((0.5, rope('k', h, qp + i)))
                for qt in range(qp, qp + PAIR):
                    for tt in range(NTT):
                        ops.append((1, v_chain(qt, tt)))

            def ak_relocate():
                # gate matmul needs ak at base partition 0 (stationary and
                # moving must share a base partition with aq)
                nc.sync.dma_start(out=st['akl'][:, :],
                                  in_=st['aqk'][c.RANK:2 * c.RANK, :])
            ops.append((0.3, ak_relocate))
            return ops, st

        def emit_attention(b, st, filler):
            credit = [0.0]

            def fill(n=1.0):
                # cost-weighted pacing: accumulate credit, pop ops while
                # affordable so big paired chains don't jam the pipeline
                credit[0] += n
                while credit[0] > 0:
                    item = next(filler, None)
                    if item is None:
                        return
                    cost, op = item
                    op()
                    credit[0] -= cost

            aq_sb = st['aqk'][0:c.RANK, :]
            ak_sb = st['akl']
            q_sb, k_sb, v_sb = st['q'], st['k'], st['v']
            og_sb = ogp.tile([128, c.HLOC, c.S], BF16, name="og_sb", tag="og")
            wo_acc = 0.0
            for qt in range(c.QTN):
                qsl = slice(qt * c.QT, (qt + 1) * c.QT)
                nkt = c.DIAG * qt + c.DIAG  # causal k tiles
                po = [pop.tile([128, c.QT], F32, name=f"po{h}", tag="po")
                      for h in range(c.HLOC)]
                # both heads' rowsums share one PSUM bank, at partitions 0
                # and 32 (separate hardware zero regions)
                prs = prsp.tile([33, c.QT], F32, name="prs", tag="prs")
                stash = {}

                def pre(kt):
                    ksl = slice(kt * c.KT, (kt + 1) * c.KT)
                    gt = None
                    if use_gate:
                        # gate tile: T = tanh(a/2); sigmoid(a) = (1+T)/2.
                        # tanh shares the exp ACT table -> no reloads.
                        pga = psp.tile([128, c.QT], F32, name="pga", tag="ps")
                        nc.tensor.matmul(pga[:, :], ak_sb[:, ksl], aq_sb[:, qsl],
                                         start=True, stop=True)
                        gt = gio.tile([128, c.QT], BF16, name="gt", tag="gt")
                        nc.scalar.activation(gt[:, :], pga[:, :], AF.Tanh,
                                             scale=0.5)
                    ptiles = []
                    for h in range(c.HLOC):
                        ps = psp.tile([128, c.QT], F32, name="ps", tag="ps")
                        nc.tensor.matmul(ps[:, :], k_sb[h][:, ksl],
                                         q_sb[h][:, qsl], start=True, stop=True)
                        p_sb = pge.tile([128, c.QT], BF16, name="p_sb", tag="p")
                        nc.scalar.activation(p_sb[:, :], ps[:, :], AF.Exp,
                                             scale=isqrt)
                        j = kt - c.DIAG * qt
                        if j >= 0 and use_mask:
                            # diagonal band: causal 0/1 mask applied AFTER exp
                            # (exp(s-1e9)=0 == exp(s)*0)
                            pm = pge.tile([128, c.QT], BF16, name="pm", tag="pm")
                            (nc.gpsimd if mask_gp else nc.vector).tensor_mul(
                                pm[:, :], p_sb[:, :], mask_sb[:, j, :])
                            p_sb = pm
                        ptiles.append(p_sb)
                    stash[kt] = (gt, ptiles)

                def cons(kt):
                    gt, ptiles = stash.pop(kt)
                    for h in range(c.HLOC):
                        p_sb = ptiles[h]
                        # rowsum (pre-gate) via ones-vector matmul; the gate's
                        # /2 is deferred to the rs copy below
                        if use_rs:
                            nc.tensor.matmul(prs[32 * h:32 * h + 1, :],
                                             ones_sb[:, 0:1], p_sb[:, :],
                                             start=(kt == 0), stop=(kt == nkt - 1))
                        if use_gate:
                            # p * (1+T) fused in one DVE op
                            pgm = pge.tile([128, c.QT], BF16, name="pgm", tag="pgm")
                            nc.vector.scalar_tensor_tensor(
                                pgm[:, :], gt[:, :], 1.0, p_sb[:, :],
                                op0=ALU.add, op1=ALU.mult)
                        else:
                            pgm = p_sb
                        # out_h^T[d, q] += v[k,d].T @ p_gated[k,q]
                        nc.tensor.matmul(po[h][:, :],
                                         v_sb[:, kt, h * 128:(h + 1) * 128],
                                         pgm[:, :],
                                         start=(kt == 0), stop=(kt == nkt - 1))

                LA = max(1, lookahead)
                for step in range(nkt + LA):
                    if step < nkt:
                        pre(step)
                    if step >= LA:
                        cons(step - LA)
                    fill()

                # normalize: og = po * (1/rowsum); rowsum scaled by 1/2 to
                # absorb the (1+T) = 2*sigmoid factor. Broadcast across
                # partitions on the idle GpSimd engine.
                for h in range(c.HLOC):
                    rs = nrm.tile([1, c.QT], F32, name="rs", tag="rs")
                    nc.scalar.mul(rs[:, :], prs[32 * h:32 * h + 1, :],
                                  2.0 if use_gate else 1.0)
                    rr = nrm.tile([1, c.QT], F32, name="rr", tag="rr")
                    if use_recip:
                        nc.vector.reciprocal_approx_fast(out=rr[:, :], in_=rs[:, :])
                    else:
                        nc.vector.tensor_copy(rr[:, :], rs[:, :])
                    rbc = nrm.tile([128, c.QT], F32, name="rbc", tag="rbc")
                    nc.gpsimd.partition_broadcast(rbc[:, :], rr[:, :])
                    nc.vector.tensor_mul(og_sb[:, h, qsl], po[h][:, :], rbc[:, :])
                    fill()

                # ---- output-projection partial for this query block:
                # pout[:, qt] = woc^T.T @ og[:, :, qt]; column chunks stage
                # into bf16 half-tiles -> 2 DMAs per query block.
                for half in range(2):
                    fq = wop.tile([128, NCH // 2, c.QT], BF16, name="fq", tag="fq")
                    for chh in range(NCH // 2):
                        ch = half * (NCH // 2) + chh
                        pf = pp.tile([128, c.QT], F32, name="pf", tag="pp")
                        for h in range(c.HLOC):
                            nc.tensor.matmul(
                                pf[:, :],
                                woc_sb[:, h, ch * 128:(ch + 1) * 128],
                                og_sb[:, h, qsl],
                                start=(h == 0), stop=(h == c.HLOC - 1))
                        # PSUM->SBUF bf16 copies split between ACT and DVE
                        wo_acc += wo_act_frac
                        if wo_acc >= 1.0:
                            wo_acc -= 1.0
                            nc.scalar.copy(fq[:, chh, :], pf[:, :])
                        else:
                            nc.vector.tensor_copy(fq[:, chh, :], pf[:, :])
                        fill()
                    pr = pout.ap().rearrange("(t p) m -> p t m", p=128)
                    nc.sync.dma_start(
                        out=pr[:, half * (NCH // 2):(half + 1) * (NCH // 2),
                               b * c.S + qt * c.QT: b * c.S + (qt + 1) * c.QT],
                        in_=fq[:, :, :])

        pend = None
        for rep in range(repeats):
            for b in range(c.B):
                ops, st = build_proj_ops(b)
                it = iter(ops)
                if pend is None:
                    for _, op in it:
                        op()
                else:
                    emit_attention(pend[0], pend[1],
                                   it if interleave else iter(()))
                    for _, op in it:  # leftovers
                        op()
                pend = (b, st)
        emit_attention(pend[0], pend[1], iter(()))


    nc.compile()
    return nc


def make_core_inputs(inputs: dict, cfg: Cfg = FULL):
    """Host-side sharding: returns in_maps (one dict per core)."""
    c = cfg
    bf16 = ml_dtypes.bfloat16
    x = np.asarray(inputs["x"])
    mask = np.asarray(inputs["mask"])
    fc = np.asarray(inputs["freqs_cos"])
    fs = np.asarray(inputs["freqs_sin"])
    wq, wk, wv, wo = (np.asarray(inputs[k]) for k in ("wq", "wk", "wv", "wo"))
    wa_q, wa_k = np.asarray(inputs["wa_q"]), np.asarray(inputs["wa_k"])

    xT = np.ascontiguousarray(x.transpose(0, 2, 1)).astype(bf16)
    waT = np.ascontiguousarray(np.concatenate([wa_q, wa_k], axis=0).T).astype(bf16)

    # rope tables in [d, tok] layout
    c2 = np.empty((c.HD, c.S), np.float32)
    s2 = np.empty((c.HD, c.S), np.float32)
    c2[0::2] = fc.T
    c2[1::2] = fc.T
    s2[0::2] = -fs.T
    s2[1::2] = fs.T
    c2 = c2.astype(bf16)
    s2 = s2.astype(bf16)

    psw = np.zeros((c.HD, c.HD), np.float32)
    idx = np.arange(c.HD)
    psw[idx, idx ^ 1] = 1.0
    psw = psw.astype(bf16)

    # diagonal-band mask patterns [j][k, q], extracted from the input mask
    qt_last = c.QTN - 1
    q0 = qt_last * c.QT
    maskd = np.empty((c.DIAG, c.KT, c.QT), np.float32)
    for j in range(c.DIAG):
        k0 = (c.DIAG * qt_last + j) * c.KT
        # multiplicative 0/1 form: positions the additive mask leaves at 0
        # (unmasked) become 1.0, masked positions (-1e9) become 0.0
        maskd[j] = (mask[0, 0, q0:q0 + c.QT, k0:k0 + c.KT].T == 0.0)
    maskd = maskd.astype(bf16)

    in_maps = []
    for ci in range(c.NCORES):
        rows = slice(ci * c.DH, (ci + 1) * c.DH)
        in_maps.append({
            "xT": xT,
            "wqT": np.ascontiguousarray(wq[rows].T).astype(bf16),
            "wkT": np.ascontiguousarray(wk[rows].T).astype(bf16),
            "wvT": np.ascontiguousarray(wv[rows].T).astype(bf16),
            "wocT": np.ascontiguousarray(wo[:, rows].T).astype(bf16),
            "waT": waT,
            "c2d": c2,
            "s2d": s2,
            "pswapd": psw,
            "maskdd": maskd,
        })
    return in_maps


def assemble_output(results, cfg: Cfg = FULL) -> np.ndarray:
    c = cfg
    total = np.zeros((c.DIM, c.B * c.S), np.float32)
    for ci in range(c.NCORES):
        total += np.asarray(results[ci]["pout"]).astype(np.float32)
    return np.ascontiguousarray(
        total.reshape(c.DIM, c.B, c.S).transpose(1, 2, 0))


_NC_CACHE = {}


def run(nc, in_maps, trace=False, cfg: Cfg = FULL, **kw):
    return bass_utils.run_bass_kernel_spmd(
        nc, in_maps, core_ids=list(range(cfg.NCORES)), trace=trace, **kw)


def kernel(**inputs) -> np.ndarray:
    cfg = FULL
    if cfg not in _NC_CACHE:
        _NC_CACHE[cfg] = build_nc(cfg)
    nc = _NC_CACHE[cfg]
    in_maps = make_core_inputs(inputs, cfg)
    res = run(nc, in_maps, cfg=cfg)
    return assemble_output(res.results, cfg)


if __name__ == "__main__":
    nc = build_nc(FULL)
    print("built ok")
